# revision 40
# baseline (speedup 1.0000x reference)
"""Multi-head self-attention Bass/Tile kernel for Trainium2, 8-core SPMD.

Problem: B=8, T=1024, C=512, H=8, HD=64.
  q/k/v = inputs @ W{q,k,v} + b          -> [B,T,H,HD]
  scores = q k^T / sqrt(HD), key-masked  -> [B,H,T,T]
  attn = softmax(scores)                  (returned as output 2)
  out  = (attn @ v @ Wo + bo) * mask     -> [B,T,C]  (output 1)

Sharding: data-parallel over batch; core b handles batch element b and
produces the full [H,T,T] attn slab plus [T,C] out rows for its batch.

Per-core design (matmuls in float32r = TF32-class, 4x faster than fp32
on the PE; elementwise/softmax math in fp32; end-to-end rel err ~6e-4):
  - x loaded natural, PE-transposed into xT [C,T] (a strided DMA load
    would generate 4-byte descriptors and run ~50x slower).
  - QT'[h] = [65,T]: rows 0-63 = (Wq.T @ xT + bq)[h]/8, row 64 = ones.
  - KTm'[h] = [65,T]: rows 0-63 = (Wk.T @ xT + bk)[h], row 64 = (1-m)*NEG.
    scores_psum = QT'.T @ KTm' then yields masked scores directly (the
    rank-1 ones x fb row adds (1-m[k])*NEG, so exp() is exactly 0 on
    masked keys). No max-subtraction needed: |scores| <= ~10 in fp32.
  - exp on ACT with accum_out = row sums (free softmax denominators);
    reciprocal + in-place normalize on DVE; attn tile DMA'd out.
  - attn normalized tiles PE-transposed 128x128 blockwise (f32r transpose
    mode, 1.5 cyc/row) and evacuated into attnT [k-major] for the attend
    contraction; attend^T[h] = V[h].T @ attnT; out = attendT.T @ Wo + bo,
    masked by query, DMA'd out.
  - 3-stage flat software pipeline over (h, qt): scores/exp -> norm/
    transpose -> evacuate, ordered so the in-order per-engine sequencers
    never head-of-line block; PSUM: 2x2-bank score slots + 2x1-bank
    transpose slots + 2-bank attend accumulator = 8 banks.
"""
import numpy as np
from contextlib import ExitStack

import concourse.bacc as bacc
import concourse.bass as bass
import concourse.mybir as mybir
from concourse import tile
from concourse.bass_utils import run_bass_kernel_spmd
from concourse.masks import make_identity

B, T, C, H = 8, 1024, 512, 8
HD = C // H
NEG = -1e30
F32 = mybir.dt.float32
F32R = mybir.dt.float32r
MULT = mybir.AluOpType.mult
ADD = mybir.AluOpType.add
EXP = mybir.ActivationFunctionType.Exp
IDENT = mybir.ActivationFunctionType.Identity

NT = T // 128   # 8 t-tiles
NCI = C // 128  # 4 ci-tiles


def build_body(tc):
    nc = tc.nc
    x_d = nc.dram_tensor("x", [T, C], F32, kind="ExternalInput").ap()
    m_d = nc.dram_tensor("mask", [T], F32, kind="ExternalInput").ap()
    w_d = {}
    for w in ("Wq", "Wk", "Wv", "Wo"):
        w_d[w] = nc.dram_tensor(w, [C, C], F32, kind="ExternalInput").ap()
    b_d = {}
    for b in ("bq", "bk", "bv", "bo"):
        b_d[b] = nc.dram_tensor(b, [C], F32, kind="ExternalInput").ap()
    out_d = nc.dram_tensor("out", [T, C], F32, kind="ExternalOutput").ap()
    attn_d = nc.dram_tensor("attn", [H, T, T], F32, kind="ExternalOutput").ap()

    ctx = ExitStack()
    with ctx:
        pers = ctx.enter_context(tc.tile_pool(name="pers", bufs=1))
        wpool = ctx.enter_context(tc.tile_pool(name="wpool", bufs=4))
        rows = ctx.enter_context(tc.tile_pool(name="rows", bufs=1))
        attnp = ctx.enter_context(tc.tile_pool(name="attnp", bufs=4))
        sumsp = ctx.enter_context(tc.tile_pool(name="sumsp", bufs=8))
        outp = ctx.enter_context(tc.tile_pool(name="outp", bufs=2))
        ps_big = ctx.enter_context(tc.tile_pool(name="ps_big", bufs=2, space="PSUM"))
        ps_tr = ctx.enter_context(tc.tile_pool(name="ps_tr", bufs=2, space="PSUM"))
        ps_att = ctx.enter_context(tc.tile_pool(name="ps_att", bufs=1, space="PSUM"))

        # ---------------- constants / loads ----------------
        ident = pers.tile([128, 128], F32, tag="ident")
        make_identity(nc, ident[:])
        ident_r = pers.tile([128, 128], F32R, tag="ident_r")
        nc.scalar.copy(ident_r[:], ident[:])

        # x natural -> PE transpose into xT [C, T]
        x_nat = [wpool.tile([128, C], F32, tag="xn", name=f"xnat{t}")
                 for t in range(NT)]
        for t in range(NT):
            nc.sync.dma_start(out=x_nat[t][:], in_=x_d[t * 128:(t + 1) * 128, :])
        xT = [pers.tile([128, T], F32R, tag="xat", name=f"xT{i}", bufs=4)
              for i in range(NCI)]
        for j in range(2):
            for ct in range(NCI):
                pt = ps_tr.tile([128, 512], F32, tag="tr", name="xtr")
                for i in range(4):
                    tt = 4 * j + i
                    nc.tensor.transpose(pt[:, i * 128:(i + 1) * 128],
                                        x_nat[tt][:, ct * 128:(ct + 1) * 128],
                                        ident[:])
                if (ct + j) % 2 == 0:
                    nc.vector.tensor_copy(xT[ct][:, j * 512:(j + 1) * 512], pt[:])
                else:
                    nc.scalar.copy(xT[ct][:, j * 512:(j + 1) * 512], pt[:])

        # mask rows
        m_row = rows.tile([1, T], F32, tag="row")          # m over keys
        nc.sync.dma_start(out=m_row[:], in_=m_d[None, :])
        fb_row = pers.tile([1, T], F32R, tag="fb")         # (1-m)*NEG
        nc.vector.tensor_scalar(out=fb_row[:], in0=m_row[:], scalar1=-NEG,
                                scalar2=NEG, op0=MULT, op1=ADD)
        maskp = pers.tile([128, NT], F32, tag="maskp")     # mask by t-partition
        nc.sync.dma_start(out=maskp[:], in_=m_d.rearrange("(a p) -> p a", p=128))

        ones_col = pers.tile([1, 128], F32, tag="ones_col")
        nc.vector.memset(ones_col[:], 1.0)
        ones_row = pers.tile([1, T], F32, tag="ones_row")
        nc.vector.memset(ones_row[:], 1.0)

        # bv/bo broadcasts [128, C]
        bvbo_bc = {}
        for name in ("bv", "bo"):
            r = rows.tile([1, C], F32, tag="row")
            nc.sync.dma_start(out=r[:], in_=b_d[name][None, :])
            t_bc = pers.tile([128, C], F32, tag=f"{name}_bc")
            ps2 = ps_big.tile([128, C], F32, tag="big")
            nc.tensor.matmul(ps2[:], ones_col[:], r[:], start=True, stop=True)
            nc.vector.tensor_copy(t_bc[:], ps2[:])
            bvbo_bc[name] = t_bc

        # per-head bias columns, batched: [64, H] (col h = bias[h*64:(h+1)*64])
        bq_all = pers.tile([64, H], F32, tag="bq_all")
        nc.sync.dma_start(out=bq_all[:], in_=b_d["bq"].rearrange("(h p) -> p h", p=64))
        nc.vector.tensor_scalar(out=bq_all[:], in0=bq_all[:], scalar1=0.125,
                                scalar2=None, op0=MULT)
        bk_all = pers.tile([64, H], F32, tag="bk_all")
        nc.sync.dma_start(out=bk_all[:], in_=b_d["bk"].rearrange("(h p) -> p h", p=64))
        bq_s = [bq_all[:, h:h + 1] for h in range(H)]
        bk_s = [bk_all[:, h:h + 1] for h in range(H)]

        # ---------------- projections ----------------
        # Q^T and K^T per head: [65, T] fp32 tiles
        QTp = [pers.tile([65, T], F32R, tag=f"QT{h}", name=f"QT{h}") for h in range(H)]
        KTm = [pers.tile([65, T], F32R, tag=f"KT{h}", name=f"KT{h}") for h in range(H)]

        for (wname, dst, proj_q) in (("Wq", QTp, True), ("Wk", KTm, False)):
            w_sb = []
            for i in range(NCI):
                wf = wpool.tile([128, C], F32, tag="w", name="wf")
                nc.sync.dma_start(out=wf[:], in_=w_d[wname][i * 128:(i + 1) * 128, :])
                wr = wpool.tile([128, C], F32R, tag="wr", name="wr")
                nc.gpsimd.tensor_copy(wr[:], wf[:])
                w_sb.append(wr)
            for ct in range(NCI):  # co tile = head pair (2ct, 2ct+1)
                ps = ps_big.tile([128, T], F32, tag="big")
                for n in range(2):
                    for ci in range(NCI):
                        nc.tensor.matmul(
                            ps[:, n * 512:(n + 1) * 512],
                            w_sb[ci][:, ct * 128:(ct + 1) * 128],
                            xT[ci][:, n * 512:(n + 1) * 512],
                            start=(ci == 0), stop=(ci == NCI - 1))
                for hh in range(2):
                    h = 2 * ct + hh
                    src = ps[hh * 64:(hh + 1) * 64, :]
                    if proj_q:
                        # (psum + bq)*0.125 == psum*0.125 + bq*0.125
                        nc.scalar.activation(out=dst[h][0:64, :], in_=src,
                                             func=IDENT, bias=bq_s[h], scale=0.125)
                        nc.vector.tensor_copy(dst[h][64:65, :], ones_row[:])
                    else:
                        # psum + bk (masking via fb row: exp(s-1e30)=0)
                        nc.scalar.activation(out=dst[h][0:64, :], in_=src,
                                             func=IDENT, bias=bk_s[h], scale=1.0)
                        nc.vector.tensor_copy(dst[h][64:65, :], fb_row[:])

        # V / Wo deferred into the first main-loop iterations (V is not
        # needed until attend(h=0) ~9 iterations in); this lets head 0's
        # scores/exp start as soon as Q/K head 0 is projected.
        V_sb = [pers.tile([128, C], F32R, tag=f"V{t}", name=f"V{t}")
                for t in range(NT)]
        Wo_r = [pers.tile([128, C], F32R, tag=f"Wo{i}", name=f"Wo{i}")
                for i in range(NCI)]
        attendT = []
        Wv_r = []

        def emit_v_loads():
            for i in range(NCI):
                wv = wpool.tile([128, C], F32, tag="w", name="wv")
                nc.sync.dma_start(out=wv[:], in_=w_d["Wv"][i * 128:(i + 1) * 128, :])
                wvr = wpool.tile([128, C], F32R, tag="wr", name="wvr")
                nc.scalar.copy(wvr[:], wv[:])
                Wv_r.append(wvr)

        def emit_v_proj(tt):
            ps = ps_big.tile([128, C], F32, tag="big", name="vproj")
            for ci in range(NCI):
                nc.tensor.matmul(ps[:], xT[ci][:, tt * 128:(tt + 1) * 128],
                                 Wv_r[ci][:], start=(ci == 0), stop=(ci == NCI - 1))
            nc.vector.scalar_tensor_tensor(
                out=V_sb[tt][:], in0=ps[:], scalar=0.0, in1=bvbo_bc["bv"][:],
                op0=ADD, op1=ADD)

        def emit_wo_attendt():
            for i in range(NCI):
                wf = wpool.tile([128, C], F32, tag="w", name="wf")
                nc.sync.dma_start(out=wf[:], in_=w_d["Wo"][i * 128:(i + 1) * 128, :])
                nc.gpsimd.tensor_copy(Wo_r[i][:], wf[:])
            for i in range(NCI):
                attendT.append(pers.tile([128, T], F32R, tag="xat",
                                         name=f"aT{i}", bufs=4))

        deferred = ([lambda: (emit_v_loads(), emit_v_proj(0))]
                    + [(lambda tt: (lambda: emit_v_proj(tt)))(t)
                       for t in range(1, NT)]
                    + [emit_wo_attendt])

        # ---------------- attention main loop ----------------
        # 3-stage flat (h, qt) software pipeline:
        #   step i:   scores(i) [PE] + exp(i) [ACT]
        #   step i+1: recip/norm(i) [DVE] + attn DMA(i) + transposes(i) [PE]
        #   step i+2: evacuate tr psum(i) -> attnT [DVE/ACT]
        # Emission order inside a step keeps every in-order engine queue's
        # head dependency already satisfied (no head-of-line blocking).
        atiles = {}
        attn_tiles = {}
        tr_psums = {}

        def emit_scores(h, qt):
            ps = ps_big.tile([128, T], F32, tag="big", name="scores")
            for n in range(2):
                nc.tensor.matmul(ps[:, n * 512:(n + 1) * 512],
                                 QTp[h][:, qt * 128:(qt + 1) * 128],
                                 KTm[h][:, n * 512:(n + 1) * 512],
                                 start=True, stop=True)
            an = attnp.tile([128, T], F32R, tag="attn", name="attn", bufs=6)
            sums = sumsp.tile([128, 1], F32, tag="sums", name="sums")
            nc.scalar.activation(out=an[:], in_=ps[:], func=EXP,
                                 accum_out=sums[:])
            attn_tiles[(h, qt)] = (an, sums)

        def emit_norm_transpose(h, qt):
            an, sums = attn_tiles[(h, qt)]
            rec = sumsp.tile([128, 1], F32, tag="recip", name="recip")
            nc.vector.reciprocal(rec[:], sums[:])
            nc.vector.tensor_scalar(out=an[:], in0=an[:], scalar1=rec[:],
                                    scalar2=None, op0=MULT)
            nc.sync.dma_start(out=attn_d[h, qt * 128:(qt + 1) * 128, :],
                              in_=an[:].bitcast(F32))
            pts = []
            for j in range(2):
                pt = ps_tr.tile([128, 512], F32R, tag="tr", name="tr")
                for i in range(4):
                    kt = 4 * j + i
                    nc.tensor.transpose(pt[:, i * 128:(i + 1) * 128],
                                        an[:, kt * 128:(kt + 1) * 128], ident_r[:])
                pts.append(pt)
            tr_psums[(h, qt)] = pts

        def emit_evacs(h, qt):
            if qt == 0:
                atile = attnp.tile([128, NT * T], F32R, tag="attnT", bufs=1,
                                   name="attnT")
                atiles[h] = atile
            atile_v = atiles[h][:].rearrange("p (k q) -> p k q", k=NT)
            attn_tiles.pop((h, qt))
            pts = tr_psums.pop((h, qt))
            for j in range(2):
                src = pts[j][:].rearrange("p (i q) -> p i q", i=4)
                dst = atile_v[:, 4 * j:4 * j + 4, qt * 128:(qt + 1) * 128]
                if j == 0 or qt % 2 == 0:
                    nc.vector.tensor_copy(dst, src)
                else:
                    nc.scalar.copy(dst, src)

        def emit_attend(h):
            atile_v = atiles[h][:].rearrange("p (k q) -> p k q", k=NT)
            pa = ps_att.tile([64, T], F32, tag="att", name="att")
            for qc in range(2):
                for kt in range(NT):
                    nc.tensor.matmul(pa[:, qc * 512:(qc + 1) * 512],
                                     V_sb[kt][:, h * 64:(h + 1) * 64],
                                     atile_v[:, kt, qc * 512:(qc + 1) * 512],
                                     start=(kt == 0), stop=(kt == NT - 1))
            dst = attendT[h // 2][(h % 2) * 64:(h % 2) * 64 + 64, :]
            if h % 2 == 0:
                nc.vector.tensor_copy(dst, pa[:])
            else:
                nc.scalar.copy(dst, pa[:])

        sched = [(h, qt) for h in range(H) for qt in range(NT)]
        for idx in range(len(sched) + 2):
            if idx >= 2:
                h_e, qt_e = sched[idx - 2]
                emit_evacs(h_e, qt_e)
                if qt_e == NT - 1:
                    emit_attend(h_e)
            if idx < len(sched):
                emit_scores(*sched[idx])
            if 1 <= idx <= len(sched):
                emit_norm_transpose(*sched[idx - 1])
            if idx < len(deferred):
                deferred[idx]()

        # ---------------- output projection (f32r) ----------------
        # Alternate psum pools (scores pool + now-idle transpose pool) for a
        # 4-deep pipeline; bias-add on DVE, mask-scale on ACT to split the
        # finalize chain across engines.
        for tt in range(NT):
            pool_ = ps_big if tt % 2 == 0 else ps_tr
            tag_ = "big" if tt % 2 == 0 else "tr"
            po = pool_.tile([128, C], F32, tag=tag_, name="oproj")
            for ci in range(NCI):
                nc.tensor.matmul(po[:], attendT[ci][:, tt * 128:(tt + 1) * 128],
                                 Wo_r[ci][:], start=(ci == 0), stop=(ci == NCI - 1))
            ot = outp.tile([128, C], F32, tag="out", name="out_sb")
            nc.vector.scalar_tensor_tensor(out=ot[:], in0=po[:], scalar=0.0,
                                           in1=bvbo_bc["bo"][:], op0=ADD, op1=ADD)
            nc.vector.tensor_scalar(out=ot[:], in0=ot[:],
                                    scalar1=maskp[:, tt:tt + 1], scalar2=None,
                                    op0=MULT)
            nc.sync.dma_start(out=out_d[tt * 128:(tt + 1) * 128, :], in_=ot[:])


_nc_cache = None


def get_nc():
    global _nc_cache
    if _nc_cache is None:
        nc = bacc.Bacc("TRN2", target_bir_lowering=False, debug=False,
                       num_devices=1)
        with tile.TileContext(nc) as tc:
            build_body(tc)
        nc.compile()
        _nc_cache = nc
    return _nc_cache


def kernel(inputs, mask, Wq, bq, Wk, bk, Wv, bv, Wo, bo):
    nc = get_nc()
    shared = {
        "Wq": np.ascontiguousarray(Wq, np.float32),
        "Wk": np.ascontiguousarray(Wk, np.float32),
        "Wv": np.ascontiguousarray(Wv, np.float32),
        "Wo": np.ascontiguousarray(Wo, np.float32),
        "bq": np.ascontiguousarray(bq, np.float32),
        "bk": np.ascontiguousarray(bk, np.float32),
        "bv": np.ascontiguousarray(bv, np.float32),
        "bo": np.ascontiguousarray(bo, np.float32),
    }
    in_maps = []
    for b in range(B):
        m = dict(shared)
        m["x"] = np.ascontiguousarray(inputs[b], np.float32)
        m["mask"] = np.ascontiguousarray(mask[b], np.float32)
        in_maps.append(m)
    res = run_bass_kernel_spmd(nc, in_maps, core_ids=list(range(B)))
    out = np.stack([res.results[b]["out"] for b in range(B)])
    attn = np.stack([res.results[b]["attn"] for b in range(B)])
    return out, attn


# revision 49
# speedup vs baseline: 1.0467x; 1.0467x over previous
"""Multi-head self-attention Bass/Tile kernel for Trainium2, 8-core SPMD.

Problem: B=8, T=1024, C=512, H=8, HD=64.
  q/k/v = inputs @ W{q,k,v} + b          -> [B,T,H,HD]
  scores = q k^T / sqrt(HD), key-masked  -> [B,H,T,T]
  attn = softmax(scores)                  (returned as output 2)
  out  = (attn @ v @ Wo + bo) * mask     -> [B,T,C]  (output 1)

Sharding: data-parallel over batch; core b handles batch element b and
produces the full [H,T,T] attn slab plus [T,C] out rows for its batch.

Per-core design (matmuls in float32r = TF32-class, 4x faster than fp32
on the PE; elementwise/softmax math in fp32; end-to-end rel err ~6e-4):
  - x loaded natural, PE-transposed into xT [C,T] (a strided DMA load
    would generate 4-byte descriptors and run ~50x slower).
  - QT'[h] = [65,T]: rows 0-63 = (Wq.T @ xT + bq)[h]/8, row 64 = ones.
  - KTm'[h] = [65,T]: rows 0-63 = (Wk.T @ xT + bk)[h], row 64 = (1-m)*NEG.
    scores_psum = QT'.T @ KTm' then yields masked scores directly (the
    rank-1 ones x fb row adds (1-m[k])*NEG, so exp() is exactly 0 on
    masked keys). No max-subtraction needed: |scores| <= ~10 in fp32.
  - exp on ACT with accum_out = row sums (free softmax denominators);
    reciprocal + in-place normalize on DVE; attn tile DMA'd out.
  - attn normalized tiles PE-transposed 128x128 blockwise (f32r transpose
    mode, 1.5 cyc/row) and evacuated into attnT [k-major] for the attend
    contraction; attend^T[h] = V[h].T @ attnT; out = attendT.T @ Wo + bo,
    masked by query, DMA'd out.
  - 3-stage flat software pipeline over (h, qt): scores/exp -> norm/
    transpose -> evacuate, ordered so the in-order per-engine sequencers
    never head-of-line block; PSUM: 2x2-bank score slots + 2x1-bank
    transpose slots + 2-bank attend accumulator = 8 banks.
"""
import numpy as np
from contextlib import ExitStack

import concourse.bacc as bacc
import concourse.bass as bass
import concourse.mybir as mybir
from concourse import tile
from concourse.bass_utils import run_bass_kernel_spmd
from concourse.masks import make_identity

B, T, C, H = 8, 1024, 512, 8
HD = C // H
NEG = -1e30
F32 = mybir.dt.float32
F32R = mybir.dt.float32r
MULT = mybir.AluOpType.mult
ADD = mybir.AluOpType.add
EXP = mybir.ActivationFunctionType.Exp
IDENT = mybir.ActivationFunctionType.Identity

NT = T // 128   # 8 t-tiles
NCI = C // 128  # 4 ci-tiles


def build_body(tc):
    nc = tc.nc
    x_d = nc.dram_tensor("x", [T, C], F32, kind="ExternalInput").ap()
    m_d = nc.dram_tensor("mask", [T], F32, kind="ExternalInput").ap()
    w_d = {}
    for w in ("Wq", "Wk", "Wv", "Wo"):
        w_d[w] = nc.dram_tensor(w, [C, C], F32, kind="ExternalInput").ap()
    b_d = {}
    for b in ("bq", "bk", "bv", "bo"):
        b_d[b] = nc.dram_tensor(b, [C], F32, kind="ExternalInput").ap()
    out_d = nc.dram_tensor("out", [T, C], F32, kind="ExternalOutput").ap()
    attn_d = nc.dram_tensor("attn", [H, T, T], F32, kind="ExternalOutput").ap()

    ctx = ExitStack()
    with ctx:
        pers = ctx.enter_context(tc.tile_pool(name="pers", bufs=1))
        wpool = ctx.enter_context(tc.tile_pool(name="wpool", bufs=4))
        rows = ctx.enter_context(tc.tile_pool(name="rows", bufs=1))
        attnp = ctx.enter_context(tc.tile_pool(name="attnp", bufs=4))
        sumsp = ctx.enter_context(tc.tile_pool(name="sumsp", bufs=8))
        outp = ctx.enter_context(tc.tile_pool(name="outp", bufs=2))
        ps_big = ctx.enter_context(tc.tile_pool(name="ps_big", bufs=2, space="PSUM"))
        ps_tr = ctx.enter_context(tc.tile_pool(name="ps_tr", bufs=3, space="PSUM"))
        ps_att = ctx.enter_context(tc.tile_pool(name="ps_att", bufs=1, space="PSUM"))

        # ---------------- constants / loads ----------------
        ident = pers.tile([128, 128], F32, tag="ident")
        make_identity(nc, ident[:])
        ident_r = pers.tile([128, 128], F32R, tag="ident_r")
        nc.scalar.copy(ident_r[:], ident[:])

        # x natural -> PE transpose into xT [C, T]
        x_nat = [wpool.tile([128, C], F32, tag="xn", name=f"xnat{t}")
                 for t in range(NT)]
        for t in range(NT):
            nc.sync.dma_start(out=x_nat[t][:], in_=x_d[t * 128:(t + 1) * 128, :])
        xT = [pers.tile([128, T], F32R, tag="xat", name=f"xT{i}", bufs=4)
              for i in range(NCI)]
        for j in range(2):
            for ct in range(NCI):
                pt = ps_tr.tile([128, 512], F32, tag="tr", name="xtr")
                for i in range(4):
                    tt = 4 * j + i
                    nc.tensor.transpose(pt[:, i * 128:(i + 1) * 128],
                                        x_nat[tt][:, ct * 128:(ct + 1) * 128],
                                        ident[:])
                if (ct + j) % 2 == 0:
                    nc.vector.tensor_copy(xT[ct][:, j * 512:(j + 1) * 512], pt[:])
                else:
                    nc.scalar.copy(xT[ct][:, j * 512:(j + 1) * 512], pt[:])

        # mask rows
        m_row = rows.tile([1, T], F32, tag="row")          # m over keys
        nc.sync.dma_start(out=m_row[:], in_=m_d[None, :])
        fb_row = pers.tile([1, T], F32R, tag="fb")         # (1-m)*NEG
        nc.vector.tensor_scalar(out=fb_row[:], in0=m_row[:], scalar1=-NEG,
                                scalar2=NEG, op0=MULT, op1=ADD)
        maskp = pers.tile([128, NT], F32, tag="maskp")     # mask by t-partition
        nc.sync.dma_start(out=maskp[:], in_=m_d.rearrange("(a p) -> p a", p=128))

        ones_col = pers.tile([1, 128], F32, tag="ones_col")
        nc.vector.memset(ones_col[:], 1.0)
        ones_row = pers.tile([1, T], F32, tag="ones_row")
        nc.vector.memset(ones_row[:], 1.0)

        # bv/bo broadcasts [128, C]
        bvbo_bc = {}
        for name in ("bv", "bo"):
            r = rows.tile([1, C], F32, tag="row")
            nc.sync.dma_start(out=r[:], in_=b_d[name][None, :])
            t_bc = pers.tile([128, C], F32, tag=f"{name}_bc")
            ps2 = ps_big.tile([128, C], F32, tag="big")
            nc.tensor.matmul(ps2[:], ones_col[:], r[:], start=True, stop=True)
            nc.vector.tensor_copy(t_bc[:], ps2[:])
            bvbo_bc[name] = t_bc

        # per-head bias columns, batched: [64, H] (col h = bias[h*64:(h+1)*64])
        bq_all = pers.tile([64, H], F32, tag="bq_all")
        nc.sync.dma_start(out=bq_all[:], in_=b_d["bq"].rearrange("(h p) -> p h", p=64))
        nc.vector.tensor_scalar(out=bq_all[:], in0=bq_all[:], scalar1=0.125,
                                scalar2=None, op0=MULT)
        bk_all = pers.tile([64, H], F32, tag="bk_all")
        nc.sync.dma_start(out=bk_all[:], in_=b_d["bk"].rearrange("(h p) -> p h", p=64))
        bq_s = [bq_all[:, h:h + 1] for h in range(H)]
        bk_s = [bk_all[:, h:h + 1] for h in range(H)]

        # ---------------- projections ----------------
        # Q^T and K^T per head: [65, T] fp32 tiles
        QTp = [pers.tile([65, T], F32R, tag=f"QT{h}", name=f"QT{h}") for h in range(H)]
        KTm = [pers.tile([65, T], F32R, tag=f"KT{h}", name=f"KT{h}") for h in range(H)]

        for (wname, dst, proj_q) in (("Wq", QTp, True), ("Wk", KTm, False)):
            w_sb = []
            for i in range(NCI):
                wf = wpool.tile([128, C], F32, tag="w", name="wf")
                nc.sync.dma_start(out=wf[:], in_=w_d[wname][i * 128:(i + 1) * 128, :])
                wr = wpool.tile([128, C], F32R, tag="wr", name="wr")
                nc.gpsimd.tensor_copy(wr[:], wf[:])
                w_sb.append(wr)
            for ct in range(NCI):  # co tile = head pair (2ct, 2ct+1)
                ps = ps_big.tile([128, T], F32, tag="big")
                for n in range(2):
                    for ci in range(NCI):
                        nc.tensor.matmul(
                            ps[:, n * 512:(n + 1) * 512],
                            w_sb[ci][:, ct * 128:(ct + 1) * 128],
                            xT[ci][:, n * 512:(n + 1) * 512],
                            start=(ci == 0), stop=(ci == NCI - 1))
                for hh in range(2):
                    h = 2 * ct + hh
                    src = ps[hh * 64:(hh + 1) * 64, :]
                    if proj_q:
                        # (psum + bq)*0.125 == psum*0.125 + bq*0.125
                        nc.scalar.activation(out=dst[h][0:64, :], in_=src,
                                             func=IDENT, bias=bq_s[h], scale=0.125)
                        nc.vector.tensor_copy(dst[h][64:65, :], ones_row[:])
                    else:
                        # psum + bk (masking via fb row: exp(s-1e30)=0)
                        nc.scalar.activation(out=dst[h][0:64, :], in_=src,
                                             func=IDENT, bias=bk_s[h], scale=1.0)
                        nc.vector.tensor_copy(dst[h][64:65, :], fb_row[:])

        # V / Wo deferred into the first main-loop iterations (V is not
        # needed until attend(h=0) ~9 iterations in); this lets head 0's
        # scores/exp start as soon as Q/K head 0 is projected.
        V_sb = [pers.tile([128, C], F32R, tag=f"V{t}", name=f"V{t}")
                for t in range(NT)]
        Wo_r = [pers.tile([128, C], F32R, tag=f"Wo{i}", name=f"Wo{i}")
                for i in range(NCI)]
        attendT = []
        Wv_r = []

        def emit_v_loads():
            for i in range(NCI):
                wv = wpool.tile([128, C], F32, tag="w", name="wv")
                nc.sync.dma_start(out=wv[:], in_=w_d["Wv"][i * 128:(i + 1) * 128, :])
                wvr = wpool.tile([128, C], F32R, tag="wr", name="wvr")
                nc.scalar.copy(wvr[:], wv[:])
                Wv_r.append(wvr)

        def emit_v_proj(tt):
            ps = ps_big.tile([128, C], F32, tag="big", name="vproj")
            for ci in range(NCI):
                nc.tensor.matmul(ps[:], xT[ci][:, tt * 128:(tt + 1) * 128],
                                 Wv_r[ci][:], start=(ci == 0), stop=(ci == NCI - 1))
            nc.vector.scalar_tensor_tensor(
                out=V_sb[tt][:], in0=ps[:], scalar=0.0, in1=bvbo_bc["bv"][:],
                op0=ADD, op1=ADD)

        def emit_wo_attendt():
            for i in range(NCI):
                wf = wpool.tile([128, C], F32, tag="w", name="wf")
                nc.sync.dma_start(out=wf[:], in_=w_d["Wo"][i * 128:(i + 1) * 128, :])
                nc.gpsimd.tensor_copy(Wo_r[i][:], wf[:])
            for i in range(NCI):
                attendT.append(pers.tile([128, T], F32R, tag="xat",
                                         name=f"aT{i}", bufs=4))

        deferred = ([lambda: (emit_v_loads(), emit_v_proj(0))]
                    + [(lambda tt: (lambda: emit_v_proj(tt)))(t)
                       for t in range(1, NT)]
                    + [emit_wo_attendt])

        # ---------------- attention main loop ----------------
        # 3-stage flat (h, qt) software pipeline:
        #   step i:   scores(i) [PE] + exp(i) [ACT]
        #   step i+1: recip/norm(i) [DVE] + attn DMA(i) + transposes(i) [PE]
        #   step i+2: evacuate tr psum(i) -> attnT [DVE/ACT]
        # Emission order inside a step keeps every in-order engine queue's
        # head dependency already satisfied (no head-of-line blocking).
        atiles = {}
        attn_tiles = {}
        tr_psums = {}

        def emit_scores(h, qt):
            ps = ps_big.tile([128, T], F32, tag="big", name="scores")
            for n in range(2):
                nc.tensor.matmul(ps[:, n * 512:(n + 1) * 512],
                                 QTp[h][:, qt * 128:(qt + 1) * 128],
                                 KTm[h][:, n * 512:(n + 1) * 512],
                                 start=True, stop=True)
            an = attnp.tile([128, T], F32R, tag="attn", name="attn", bufs=6)
            sums = sumsp.tile([128, 1], F32, tag="sums", name="sums")
            nc.scalar.activation(out=an[:], in_=ps[:], func=EXP,
                                 accum_out=sums[:])
            attn_tiles[(h, qt)] = (an, sums)

        def emit_norm_transpose(h, qt):
            an, sums = attn_tiles[(h, qt)]
            rec = sumsp.tile([128, 1], F32, tag="recip", name="recip")
            nc.vector.reciprocal(rec[:], sums[:])
            nc.vector.tensor_scalar(out=an[:], in0=an[:], scalar1=rec[:],
                                    scalar2=None, op0=MULT)
            nc.sync.dma_start(out=attn_d[h, qt * 128:(qt + 1) * 128, :],
                              in_=an[:].bitcast(F32))
            pts = []
            for j in range(2):
                pt = ps_tr.tile([128, 512], F32R, tag="tr", name="tr")
                for i in range(4):
                    kt = 4 * j + i
                    nc.tensor.transpose(pt[:, i * 128:(i + 1) * 128],
                                        an[:, kt * 128:(kt + 1) * 128], ident_r[:])
                pts.append(pt)
            tr_psums[(h, qt)] = pts

        def emit_evacs(h, qt):
            if qt == 0:
                atile = attnp.tile([128, NT * T], F32R, tag="attnT", bufs=1,
                                   name="attnT")
                atiles[h] = atile
            atile_v = atiles[h][:].rearrange("p (k q) -> p k q", k=NT)
            attn_tiles.pop((h, qt))
            pts = tr_psums.pop((h, qt))
            for j in range(2):
                src = pts[j][:].rearrange("p (i q) -> p i q", i=4)
                dst = atile_v[:, 4 * j:4 * j + 4, qt * 128:(qt + 1) * 128]
                if j == 0 or qt % 2 == 0:
                    nc.vector.tensor_copy(dst, src)
                else:
                    nc.scalar.copy(dst, src)

        def emit_attend(h):
            atile_v = atiles[h][:].rearrange("p (k q) -> p k q", k=NT)
            for qc in range(2):
                pa = ps_att.tile([64, 512], F32, tag="att", name="att", bufs=1)
                for kt in range(NT):
                    nc.tensor.matmul(pa[:],
                                     V_sb[kt][:, h * 64:(h + 1) * 64],
                                     atile_v[:, kt, qc * 512:(qc + 1) * 512],
                                     start=(kt == 0), stop=(kt == NT - 1))
                dst = attendT[h // 2][(h % 2) * 64:(h % 2) * 64 + 64,
                                      qc * 512:(qc + 1) * 512]
                if (h + qc) % 2 == 0:
                    nc.vector.tensor_copy(dst, pa[:])
                else:
                    nc.scalar.copy(dst, pa[:])

        sched = [(h, qt) for h in range(H) for qt in range(NT)]
        for idx in range(len(sched) + 3):
            if idx >= 3:
                h_e, qt_e = sched[idx - 3]
                emit_evacs(h_e, qt_e)
                if qt_e == NT - 1:
                    emit_attend(h_e)
            if idx < len(sched):
                emit_scores(*sched[idx])
            if 1 <= idx <= len(sched):
                emit_norm_transpose(*sched[idx - 1])
            if idx < len(deferred):
                deferred[idx]()

        # ---------------- output projection (f32r) ----------------
        # Alternate psum pools (scores pool + now-idle transpose pool) for a
        # 4-deep pipeline; bias-add on DVE, mask-scale on ACT to split the
        # finalize chain across engines.
        for tt in range(NT):
            pool_ = ps_big if tt % 2 == 0 else ps_tr
            tag_ = "big" if tt % 2 == 0 else "tr"
            po = pool_.tile([128, C], F32, tag=tag_, name="oproj")
            for ci in range(NCI):
                nc.tensor.matmul(po[:], attendT[ci][:, tt * 128:(tt + 1) * 128],
                                 Wo_r[ci][:], start=(ci == 0), stop=(ci == NCI - 1))
            ot = outp.tile([128, C], F32, tag="out", name="out_sb")
            nc.vector.scalar_tensor_tensor(out=ot[:], in0=po[:], scalar=0.0,
                                           in1=bvbo_bc["bo"][:], op0=ADD, op1=ADD)
            nc.vector.tensor_scalar(out=ot[:], in0=ot[:],
                                    scalar1=maskp[:, tt:tt + 1], scalar2=None,
                                    op0=MULT)
            nc.sync.dma_start(out=out_d[tt * 128:(tt + 1) * 128, :], in_=ot[:])


_nc_cache = None


def get_nc():
    global _nc_cache
    if _nc_cache is None:
        nc = bacc.Bacc("TRN2", target_bir_lowering=False, debug=False,
                       num_devices=1)
        with tile.TileContext(nc) as tc:
            build_body(tc)
        nc.compile()
        _nc_cache = nc
    return _nc_cache


def kernel(inputs, mask, Wq, bq, Wk, bk, Wv, bv, Wo, bo):
    nc = get_nc()
    shared = {
        "Wq": np.ascontiguousarray(Wq, np.float32),
        "Wk": np.ascontiguousarray(Wk, np.float32),
        "Wv": np.ascontiguousarray(Wv, np.float32),
        "Wo": np.ascontiguousarray(Wo, np.float32),
        "bq": np.ascontiguousarray(bq, np.float32),
        "bk": np.ascontiguousarray(bk, np.float32),
        "bv": np.ascontiguousarray(bv, np.float32),
        "bo": np.ascontiguousarray(bo, np.float32),
    }
    in_maps = []
    for b in range(B):
        m = dict(shared)
        m["x"] = np.ascontiguousarray(inputs[b], np.float32)
        m["mask"] = np.ascontiguousarray(mask[b], np.float32)
        in_maps.append(m)
    res = run_bass_kernel_spmd(nc, in_maps, core_ids=list(range(B)))
    out = np.stack([res.results[b]["out"] for b in range(B)])
    attn = np.stack([res.results[b]["attn"] for b in range(B)])
    return out, attn
